# revision 1
# baseline (speedup 1.0000x reference)
"""Max pairwise L2 distance between two embedding sets, on 8 Trainium2 cores.

Problem: l [8192, 64] f32, r [8192, 64] f32 -> scalar f32
    out = sqrt(max_ij ||l_i - r_j||^2)

Strategy
--------
The distance matrix has 67M entries; any exact max must examine every one.
On TRN2 the only engines that can read PSUM (where matmul output lands) are
VectorE (1 fp32/lane/cycle @ 0.96 GHz) and ScalarE (1/lane/cycle @ 1.2 GHz),
so the examination is the bottleneck, not the matmul.  We therefore:

1. On host, pick a strong candidate pair (extreme norms / extreme projections)
   and compute its exact distance L.  Pick thr = L - delta where delta bounds
   the bf16 matmul error.  Any entry <= thr cannot beat L.
2. Augment the K dimension so the PE itself computes sq_dist - thr:
      l_aug = [-2*l | lsq_hi lsq_lo 1 1 1]       (K = 69 rows, bf16)
      r_aug = [  r  | 1 1 rsq_hi rsq_lo -thr]
   (norms carried as bf16 hi+lo pairs for accuracy; thr exactly bf16).
3. Shard rows of l across the 8 cores (1024 each); every core streams all of
   r.  Each core runs 128 matmuls of [69,128]x[69,512], two per 2-bank PSUM
   group (64 groups, 4-deep rotation); ScalarE relu+sum-accumulates 36
   groups, VectorE max-reduces 28 (split matches the engines' measured
   per-group cost, ACT ~1.04us vs DVE ~1.19us; imbalance stalls the PSUM
   ring, idles the PE, and triggers HAM re-throttle).  Groups are ordered
   m-tile-major so the PE stationary only changes every 8th group; redundant
   InstLdweights are stripped before compile (the PE array keeps the loaded
   stationary across matmuls).  A partition-row's partial > 0 iff some entry
   exceeded thr.
4. Host exactly (float64) recomputes the few flagged rows and returns
   sqrt(max(L, flagged maxima)) - an exact fp32 answer.
"""

import numpy as np
import ml_dtypes

N_CORES = 8
N_L, N_R, DIM = 8192, 8192, 64
K_AUG = 69                      # 64 dims + lsq_hi/lo + rsq_hi/lo + thr
L_COLS = N_L // N_CORES         # 1024 l-rows per core
M_TILE = 128                    # stationary free dim (l rows per matmul)
N_FREE = 256                    # moving free dim (half a PSUM bank; 4 MMs
                                # per group interleave the PE stream more
                                # finely against ring waits — measured ahead
                                # of 512 in two paired windows)
CHUNK = 1024                    # consumer group = 2 banks
N_ACT = 36                      # groups consumed by ScalarE (of 64)
# ACT/DVE interleave, tiled over the 64 groups.  Leading with ScalarE (the
# faster consumer) and placing the doubled-A at ring positions 0-1 measured
# consistently ahead of the Bresenham spread in paired trials; dropping the
# 4th period's doubled-A (35 ACT / 29 DVE) rebalances the slightly
# ACT-heavy split and measured ahead again.
ASSIGN_PATTERN = ("ADADADADAADADADA" * 3 + "ADADADADADADADAD")
BF16 = ml_dtypes.bfloat16

_COMPILED = {}


def _assignment(groups, n_act=N_ACT, pattern=None):
    """ACT/DVE split. Returns list of bools (True = ACT).

    pattern=None: Bresenham-spread n_act ACT groups among `groups`.
    pattern=str of 'A'/'D': tiled verbatim (n_act ignored)."""
    if pattern is not None:
        out = [pattern[g % len(pattern)] == "A" for g in range(groups)]
        return out
    out = []
    acc = 0
    for _ in range(groups):
        acc += n_act
        if acc >= groups:
            acc -= groups
            out.append(True)
        else:
            out.append(False)
    assert sum(out) == n_act
    return out


def _dedup_ldweights(nc):
    """Drop InstLdweights whose weights match the previous LDW in the same
    basic block.  The matmuls still carry the weights AP (dep tracking
    intact); the PE array keeps the loaded stationary across matmuls, so a
    reload with identical weights is pure overhead (~107ns engine time each).
    Only sync-free LDWs are dropped (waits/updates must survive)."""
    removed = 0
    for fn in nc.m.functions:
        for blk in fn.blocks:
            insts = list(blk.instructions)
            last_sig = None
            keep = []
            for inst in insts:
                if type(inst).__name__ == "InstLdweights":
                    si = inst.sync_info
                    clean = si is None or (
                        not list(si.on_wait) and not list(si.on_update))
                    sig = str(inst.ins[-1])
                    if sig == last_sig and clean:
                        removed += 1
                        continue
                    last_sig = sig
                keep.append(inst)
            if len(keep) != len(insts):
                blk.instructions = keep
    return removed


def _build_nc(l_cols=L_COLS, r_cols=N_R, repeats=1, dyn_loop=False,
              consumer="mixed", chunk=CHUNK, psum_bufs=4, n_act=N_ACT,
              dedup_ldw=True, order="m_major", act_accum_psum=False,
              n_free=N_FREE, mm_start=True, dummy_free=0,
              assign_pattern=ASSIGN_PATTERN):
    """Build + compile the per-core SPMD program.

    Inputs : l_blk [K_AUG, l_cols] bf16, r_all [K_AUG, r_cols] bf16
             (+ rep_cnt [1,1] i32 when dyn_loop)
    Outputs: dve_part [128, n_dve] f32  (max of sq-thr over group)
             act_part [128, n_act] f32  (sum of relu(sq-thr))

    dyn_loop=True wraps the group loop in a runtime-count For_i (for
    timing: one NEFF, variable work).
    """
    import concourse.tile as tile
    from concourse import bacc, mybir
    from concourse.bass import make_scalar_value, RegisterHandles

    m_tiles = l_cols // M_TILE
    n_chunks = r_cols // chunk
    groups = m_tiles * n_chunks
    if consumer == "mixed":
        assign_act = _assignment(groups, n_act, pattern=assign_pattern)
    elif consumer == "act":
        assign_act = [True] * groups
    elif consumer in ("dve", "none"):
        assign_act = [False] * groups
    else:
        raise ValueError(consumer)
    n_act_eff = sum(assign_act)
    n_dve = groups - n_act_eff

    nc = bacc.Bacc("TRN2", target_bir_lowering=False, debug=False,
                   num_devices=N_CORES)
    bf16 = mybir.dt.bfloat16
    f32 = mybir.dt.float32

    l_in = nc.dram_tensor("l_blk", [K_AUG, l_cols], bf16,
                          kind="ExternalInput").ap()
    r_in = nc.dram_tensor("r_all", [K_AUG, r_cols], bf16,
                          kind="ExternalInput").ap()
    cnt_in = None
    if dyn_loop:
        cnt_in = nc.dram_tensor("rep_cnt", [1, 1], mybir.dt.int32,
                                kind="ExternalInput").ap()
    dve_out = act_out = None
    if n_dve:
        dve_out = nc.dram_tensor("dve_part", [128, n_dve], f32,
                                 kind="ExternalOutput").ap()
    if n_act_eff:
        act_out = nc.dram_tensor("act_part", [128, n_act_eff], f32,
                                 kind="ExternalOutput").ap()

    import contextlib
    with tile.TileContext(nc) as tc:
        with (tc.tile_pool(name="io", bufs=1) as io_pool,
              tc.tile_pool(name="psum", bufs=psum_bufs, space="PSUM") as psum_pool,
              (tc.tile_pool(name="acc", bufs=1, space="PSUM")
               if act_accum_psum else contextlib.nullcontext()) as acc_pool,
              (tc.tile_pool(name="dummy", bufs=1, space="PSUM")
               if dummy_free else contextlib.nullcontext()) as dummy_pool,
              tc.tile_pool(name="scratch", bufs=1) as scratch_pool):
            # tiny dummy activation first so the ACT table set loads during
            # the DMA prologue instead of before the first real group
            warm = scratch_pool.tile([128, 1], f32)
            nc.vector.memset(warm[:], 0.0)
            nc.scalar.activation(warm[:], warm[:],
                                 mybir.ActivationFunctionType.Relu)

            l_sb = io_pool.tile([K_AUG, l_cols], bf16)
            # first m-tile lands first so group 0 starts ASAP
            nc.sync.dma_start(l_sb[:, :M_TILE], l_in[:, :M_TILE])
            nc.sync.dma_start(l_sb[:, M_TILE:], l_in[:, M_TILE:])
            r_sb = io_pool.tile([K_AUG, r_cols], bf16)
            for ch in range(n_chunks):
                sl = slice(ch * chunk, (ch + 1) * chunk)
                nc.sync.dma_start(r_sb[:, sl], r_in[:, sl])

            dve_sb = act_sb = act_acc = None
            if n_dve:
                dve_sb = io_pool.tile([128, n_dve], f32, name="dve_sb")
            if n_act_eff:
                act_sb = io_pool.tile([128, n_act_eff], f32, name="act_sb")
                act_acc = (acc_pool.tile([128, n_act_eff], f32, name="act_acc")
                           if act_accum_psum else act_sb)
            if consumer == "none" and dve_sb is not None:
                nc.vector.memset(dve_sb[:], 0.0)

            dummy_ps = (dummy_pool.tile([128, dummy_free], f32,
                                        name="dummy_ps") if dummy_free else None)

            def body():
                dve_slot = 0
                act_slot = 0
                for g in range(groups):
                    if order == "m_major":
                        # stationary constant across n_chunks consecutive
                        # groups -> LDW dedup strips the reloads
                        m, ch = divmod(g, n_chunks)
                    else:
                        ch, m = divmod(g, m_tiles)
                    ps = psum_pool.tile([128, chunk], f32)
                    for k in range(chunk // n_free):
                        ncol = ch * chunk + k * n_free
                        nc.tensor.matmul(
                            ps[:, k * n_free:(k + 1) * n_free],
                            l_sb[:, m * M_TILE:(m + 1) * M_TILE],
                            r_sb[:, ncol:ncol + n_free],
                            start=mm_start, stop=True,
                            skip_group_check=not mm_start)
                    if dummy_ps is not None:
                        # keep-warm filler: unread matmul into a spare bank so
                        # the PE never idles long enough for HAM to re-throttle
                        nc.tensor.matmul(
                            dummy_ps[:, :],
                            l_sb[:, m * M_TILE:(m + 1) * M_TILE],
                            r_sb[:, :dummy_free],
                            start=True, stop=True)
                    if consumer == "none":
                        continue
                    if assign_act[g]:
                        # relu in place in PSUM (ScalarE is closest to PSUM;
                        # next matmul start=True clears has_written anyway)
                        nc.scalar.activation(
                            ps[:, :], ps[:, :],
                            mybir.ActivationFunctionType.Relu,
                            accum_out=act_acc[:, act_slot:act_slot + 1])
                        act_slot += 1
                    else:
                        nc.vector.tensor_reduce(
                            dve_sb[:, dve_slot:dve_slot + 1], ps[:, :],
                            axis=mybir.AxisListType.X, op=mybir.AluOpType.max)
                        dve_slot += 1

            if dyn_loop:
                cnt_sb = io_pool.tile([1, 1], mybir.dt.int32)
                nc.sync.dma_start(cnt_sb[:], cnt_in[:])
                regs = []
                for etype in mybir.ALL_ENGINES:
                    eng = nc.engines[etype]
                    reg = eng.alloc_register(f"repcnt_{etype.name}")
                    eng.reg_load(reg, cnt_sb[0:1, 0:1])
                    regs.append(reg)
                end_sv = make_scalar_value(
                    RegisterHandles(regs), min_val=0, max_val=100000)
                with tc.For_i(0, end_sv):
                    for _ in range(repeats):
                        body()
            else:
                for _ in range(repeats):
                    body()

            if dve_out is not None:
                nc.sync.dma_start(dve_out[:], dve_sb[:])
            if act_out is not None:
                if act_acc is not act_sb:
                    # DMA cannot read PSUM; stage through SBUF
                    nc.scalar.copy(act_sb[:], act_acc[:])
                nc.sync.dma_start(act_out[:], act_sb[:])

    if dedup_ldw:
        _dedup_ldweights(nc)
    nc.compile()
    return nc


def _get_nc(key=("full", 1)):
    if key not in _COMPILED:
        kind, repeats = key
        _COMPILED[key] = _build_nc(repeats=repeats)
    return _COMPILED[key]


def _candidate_threshold(l64, r64, ln, rn):
    """Exact (float64) max squared distance over a cheap candidate set."""
    cl = set(np.argsort(-ln)[:64].tolist())
    cr = set(np.argsort(-rn)[:64].tolist())
    rng = np.random.default_rng(12345)
    U = rng.standard_normal((16, DIM))
    U /= np.linalg.norm(U, axis=1, keepdims=True)
    pl = l64 @ U.T
    pr = r64 @ U.T
    for k in range(U.shape[0]):
        cl.update(np.argsort(-pl[:, k])[:8].tolist())
        cl.update(np.argsort(pl[:, k])[:8].tolist())
        cr.update(np.argsort(-pr[:, k])[:8].tolist())
        cr.update(np.argsort(pr[:, k])[:8].tolist())
    A = l64[sorted(cl)]
    B = r64[sorted(cr)]
    d2 = ((A * A).sum(1)[:, None] + (B * B).sum(1)[None, :]
          - 2.0 * (A @ B.T))
    return float(d2.max())


def _hi_lo_bf16(x64):
    hi = x64.astype(np.float32).astype(BF16)
    lo = (x64 - hi.astype(np.float64)).astype(np.float32).astype(BF16)
    return hi, lo


def _prepare_inputs(l, r):
    """Returns (l_aug [K_AUG, N_L] bf16, r_aug [K_AUG, N_R] bf16, L, thr)."""
    l64 = l.astype(np.float64)
    r64 = r.astype(np.float64)
    lsq = (l64 * l64).sum(1)
    rsq = (r64 * r64).sum(1)
    ln = np.sqrt(lsq)
    rn = np.sqrt(rsq)

    L = _candidate_threshold(l64, r64, ln, rn)
    # bf16 error bound on device sq-dist: cross term 2^-8 * 2*|l||r|, plus
    # slack for norm hi/lo rounding and fp32 accumulation.
    delta = 2.0 ** -8 * 2.0 * float(ln.max()) * float(rn.max()) + 0.05
    thr = float(np.asarray(L - delta, dtype=np.float32).astype(BF16))

    lsq_hi, lsq_lo = _hi_lo_bf16(lsq)
    rsq_hi, rsq_lo = _hi_lo_bf16(rsq)

    l_aug = np.zeros((K_AUG, N_L), dtype=BF16)
    l_aug[:DIM] = (-2.0 * l.astype(np.float32).T).astype(BF16)
    l_aug[64] = lsq_hi
    l_aug[65] = lsq_lo
    l_aug[66] = BF16(1.0)
    l_aug[67] = BF16(1.0)
    l_aug[68] = BF16(1.0)

    r_aug = np.zeros((K_AUG, N_R), dtype=BF16)
    r_aug[:DIM] = r.astype(np.float32).T.astype(BF16)
    r_aug[64] = BF16(1.0)
    r_aug[65] = BF16(1.0)
    r_aug[66] = rsq_hi
    r_aug[67] = rsq_lo
    r_aug[68] = BF16(-thr)

    return np.ascontiguousarray(l_aug), np.ascontiguousarray(r_aug), L, thr


def _run_device(l_aug, r_aug, nc=None):
    from concourse.bass_utils import run_bass_kernel_spmd
    if nc is None:
        nc = _get_nc()
    in_maps = [
        {"l_blk": np.ascontiguousarray(l_aug[:, c * L_COLS:(c + 1) * L_COLS]),
         "r_all": r_aug}
        for c in range(N_CORES)
    ]
    res = run_bass_kernel_spmd(nc, in_maps, core_ids=list(range(N_CORES)))
    return res.results


def kernel(l_dfa_embeddings, r_dfa_embeddings):
    l = np.asarray(l_dfa_embeddings, dtype=np.float32)
    r = np.asarray(r_dfa_embeddings, dtype=np.float32)
    assert l.shape == (N_L, DIM) and r.shape == (N_R, DIM)

    l_aug, r_aug, L, thr = _prepare_inputs(l, r)
    results = _run_device(l_aug, r_aug)

    l64 = l.astype(np.float64)
    r64 = r.astype(np.float64)
    rsq = (r64 * r64).sum(1)

    m_tiles = L_COLS // M_TILE
    n_chunks = N_R // CHUNK
    groups = m_tiles * n_chunks
    assign_act = _assignment(groups, pattern=ASSIGN_PATTERN)
    best = L
    for c in range(N_CORES):
        dve = results[c].get("dve_part")
        act = results[c].get("act_part")
        dve_slot = act_slot = 0
        for g in range(groups):
            if assign_act[g]:
                part = act[:, act_slot]
                act_slot += 1
            else:
                part = dve[:, dve_slot]
                dve_slot += 1
            flagged = np.nonzero(part > 0.0)[0]
            if flagged.size == 0:
                continue
            m, ch = divmod(g, n_chunks)
            cols = slice(ch * CHUNK, (ch + 1) * CHUNK)
            for p in flagged:
                lrow = c * L_COLS + m * M_TILE + int(p)
                d2 = ((l64[lrow] * l64[lrow]).sum() + rsq[cols]
                      - 2.0 * (r64[cols] @ l64[lrow]))
                best = max(best, float(d2.max()))

    return np.float32(np.sqrt(max(best, 0.0)))



# revision 4
# speedup vs baseline: 7.2270x; 7.2270x over previous
"""Max pairwise L2 distance between two embedding sets, on 8 Trainium2 cores.

Problem: l [8192, 64] f32, r [8192, 64] f32 -> scalar f32
    out = sqrt(max_ij ||l_i - r_j||^2)

Strategy (v2: group-bound certificate)
--------------------------------------
The distance matrix has 67M entries.  On TRN2 only VectorE/ScalarE can read
PSUM (1 fp32/lane/cycle), so exhaustive per-pair examination costs ~35us.
Instead each PSUM entry certifies a whole GROUP of r-points via a provable
upper bound.  For a group G with center c, radius rad = max_j ||r_j - c||,
s = max_j ||r_j||^2:

    max_{j in G} ||l_i - r_j||^2 <= lsq_i + s - 2 l_i.c + 2 ||l_i|| rad

which is a single dot product of augmented vectors:
    rg_vec = [-2c (64) | 1 | 2*rad | s - thr]      (stationary, bf16)
    l_vec  = [l_i (64) | lsq_i | ||l_i|| | 1]      (moving, bf16)

1. Host picks a candidate max L over extreme-norm/extreme-projection pairs
   and sets thr = L - delta (delta rigorously bounds bf16 rounding).
2. Host groups r-points by "heat" (estimated max distance to any l, via the
   candidate l-set): hot points get tiny groups (size 1-2, tight bound),
   the quiet bulk gets coarse groups (4-16).  NG total groups.
3. Device: r-group vectors are the stationary operand (NG/8 = blocks of 128
   per core), all 8192 l-columns stream as moving.  ScalarE relu+accum and
   VectorE max-reduce consume PSUM; a positive partial flags (group,
   l-chunk) cells.  l-rows are heat-sorted so flags concentrate in few
   cells.
4. Host rechecks flagged cells exactly (float64) and returns
   sqrt(max(L, flagged maxima)) - an exact fp32 answer for ANY input;
   grouping quality only affects speed.
"""

import numpy as np
import ml_dtypes

N_CORES = 8
N_L, N_R, DIM = 8192, 8192, 64
K_AUG = 67                      # 64 dims + lsq/ln/1 rows
NG = 1024                       # total r-groups (multiple of 128*N_CORES)
N_BLOCKS = NG // (128 * N_CORES)
LCOLS = N_L                     # every core streams all l columns
CHUNK = 1024                    # l-cols consumed per PSUM op
MM_FREE = 512                   # moving free dim per matmul (1 PSUM bank)
PSUM_BUFS = 3
N_OPS = N_BLOCKS * (LCOLS // CHUNK)
N_ACT = 4                       # ScalarE ops of the N_OPS (rest VectorE)
# hottest r-points first: (count, group_size); counts*sizes must sum to N_R
QUOTA = ((400, 1), (8, 2), (56, 4), (176, 8), (384, 16))
BF16 = ml_dtypes.bfloat16

_COMPILED = {}


def _assignment(n_ops=N_OPS, n_act=N_ACT):
    """Bresenham-spread n_act ScalarE ops among n_ops. True = ACT."""
    out = []
    acc = 0
    for _ in range(n_ops):
        acc += n_act
        if acc >= n_ops:
            acc -= n_ops
            out.append(True)
        else:
            out.append(False)
    assert sum(out) == n_act
    return out


def _dedup_ldweights(nc):
    """Drop InstLdweights whose weights match the previous LDW in the same
    basic block (the PE keeps the loaded stationary across matmuls)."""
    removed = 0
    for fn in nc.m.functions:
        for blk in fn.blocks:
            insts = list(blk.instructions)
            last_sig = None
            keep = []
            for inst in insts:
                if type(inst).__name__ == "InstLdweights":
                    si = inst.sync_info
                    clean = si is None or (
                        not list(si.on_wait) and not list(si.on_update))
                    sig = str(inst.ins[-1])
                    if sig == last_sig and clean:
                        removed += 1
                        continue
                    last_sig = sig
                keep.append(inst)
            if len(keep) != len(insts):
                blk.instructions = keep
    return removed


def _build_nc(n_blocks=N_BLOCKS, lcols=LCOLS, chunk=CHUNK, n_act=N_ACT,
              mm_free=MM_FREE, psum_bufs=PSUM_BUFS, dyn_loop=False,
              repeats=1, dedup_ldw=True):
    """Build + compile the per-core SPMD program.

    Inputs : rg_all [K_AUG, n_blocks*128] bf16, l_all [K_AUG, lcols] bf16
             (+ rep_cnt [1,1] i32 when dyn_loop)
    Outputs: dve_part [128, n_dve] f32 (max of bound-thr over chunk)
             act_part [128, n_act] f32 (sum of relu(bound-thr))
    """
    import concourse.tile as tile
    from concourse import bacc, mybir
    from concourse.bass import make_scalar_value, RegisterHandles

    n_ops = n_blocks * (lcols // chunk)
    assign_act = _assignment(n_ops, n_act)
    n_dve = n_ops - n_act
    ngc = n_blocks * 128

    nc = bacc.Bacc("TRN2", target_bir_lowering=False, debug=False,
                   num_devices=N_CORES)
    bf16 = mybir.dt.bfloat16
    f32 = mybir.dt.float32

    rg_in = nc.dram_tensor("rg_all", [K_AUG, ngc], bf16,
                           kind="ExternalInput").ap()
    l_in = nc.dram_tensor("l_all", [K_AUG, lcols], bf16,
                          kind="ExternalInput").ap()
    cnt_in = None
    if dyn_loop:
        cnt_in = nc.dram_tensor("rep_cnt", [1, 1], mybir.dt.int32,
                                kind="ExternalInput").ap()
    dve_out = act_out = None
    if n_dve:
        dve_out = nc.dram_tensor("dve_part", [128, n_dve], f32,
                                 kind="ExternalOutput").ap()
    if n_act:
        act_out = nc.dram_tensor("act_part", [128, n_act], f32,
                                 kind="ExternalOutput").ap()

    with tile.TileContext(nc) as tc:
        with (tc.tile_pool(name="io", bufs=1) as io_pool,
              tc.tile_pool(name="psum", bufs=psum_bufs, space="PSUM") as psum_pool,
              tc.tile_pool(name="scratch", bufs=1) as scratch_pool):
            # tiny dummy activation first so the ACT table set loads during
            # the DMA prologue instead of before the first real op
            warm = scratch_pool.tile([128, 1], f32)
            nc.vector.memset(warm[:], 0.0)
            nc.scalar.activation(warm[:], warm[:],
                                 mybir.ActivationFunctionType.Relu)

            rg_sb = io_pool.tile([K_AUG, ngc], bf16)
            nc.sync.dma_start(rg_sb[:], rg_in[:])
            l_sb = io_pool.tile([K_AUG, lcols], bf16)
            # first chunk lands first so op 0 starts ASAP
            nc.sync.dma_start(l_sb[:, :chunk], l_in[:, :chunk])
            for c0 in range(chunk, lcols, chunk):
                nc.sync.dma_start(l_sb[:, c0:c0 + chunk],
                                  l_in[:, c0:c0 + chunk])

            dve_sb = act_sb = None
            if n_dve:
                dve_sb = io_pool.tile([128, n_dve], f32, name="dve_sb")
            if n_act:
                act_sb = io_pool.tile([128, n_act], f32, name="act_sb")

            def body():
                dve_slot = 0
                act_slot = 0
                op = 0
                for b in range(n_blocks):
                    stat = rg_sb[:, b * 128:(b + 1) * 128]
                    for ch in range(lcols // chunk):
                        ps = psum_pool.tile([128, chunk], f32)
                        for k in range(chunk // mm_free):
                            ncol = ch * chunk + k * mm_free
                            nc.tensor.matmul(
                                ps[:, k * mm_free:(k + 1) * mm_free],
                                stat, l_sb[:, ncol:ncol + mm_free],
                                start=True, stop=True)
                        if assign_act[op]:
                            nc.scalar.activation(
                                ps[:, :], ps[:, :],
                                mybir.ActivationFunctionType.Relu,
                                accum_out=act_sb[:, act_slot:act_slot + 1])
                            act_slot += 1
                        else:
                            nc.vector.tensor_reduce(
                                dve_sb[:, dve_slot:dve_slot + 1], ps[:, :],
                                axis=mybir.AxisListType.X,
                                op=mybir.AluOpType.max)
                            dve_slot += 1
                        op += 1

            if dyn_loop:
                cnt_sb = io_pool.tile([1, 1], mybir.dt.int32)
                nc.sync.dma_start(cnt_sb[:], cnt_in[:])
                regs = []
                for etype in mybir.ALL_ENGINES:
                    eng = nc.engines[etype]
                    reg = eng.alloc_register(f"repcnt_{etype.name}")
                    eng.reg_load(reg, cnt_sb[0:1, 0:1])
                    regs.append(reg)
                end_sv = make_scalar_value(
                    RegisterHandles(regs), min_val=0, max_val=100000)
                with tc.For_i(0, end_sv):
                    for _ in range(repeats):
                        body()
            else:
                for _ in range(repeats):
                    body()

            if dve_out is not None:
                nc.sync.dma_start(dve_out[:], dve_sb[:])
            if act_out is not None:
                nc.sync.dma_start(act_out[:], act_sb[:])

    if dedup_ldw:
        _dedup_ldweights(nc)
    nc.compile()
    return nc


def _get_nc(key=("full", 1)):
    if key not in _COMPILED:
        kind, repeats = key
        _COMPILED[key] = _build_nc(repeats=repeats)
    return _COMPILED[key]


# ---------------------------------------------------------------- host side

def _candidate_set(x64, xn, nrm_top=96, nproj=24, proj_top=8, seed=777):
    """Indices of extreme-norm / extreme-projection points."""
    cs = set(np.argsort(-xn)[:nrm_top].tolist())
    rng = np.random.default_rng(seed)
    U = rng.standard_normal((nproj, DIM))
    U /= np.linalg.norm(U, axis=1, keepdims=True)
    p = x64 @ U.T
    for k in range(nproj):
        cs.update(np.argsort(-p[:, k])[:proj_top].tolist())
        cs.update(np.argsort(p[:, k])[:proj_top].tolist())
    return np.array(sorted(cs))


def _est_heat(targets, cand_pts):
    """max_i d2(cand_i, target_j) for each target row (float64)."""
    tsq = (targets * targets).sum(1)
    csq = (cand_pts * cand_pts).sum(1)
    d2 = csq[:, None] + tsq[None, :] - 2.0 * (cand_pts @ targets.T)
    return d2.max(axis=0)


def _greedy_pair(pts):
    """Greedy min-distance matching of pts [n, d] -> [n//2, 2] local idx."""
    n = pts.shape[0]
    sq = (pts * pts).sum(1)
    d2 = sq[:, None] + sq[None, :] - 2.0 * (pts @ pts.T)
    np.fill_diagonal(d2, np.inf)
    k = min(8, n - 1)
    nbr = np.argpartition(d2, k - 1, axis=1)[:, :k]
    w = np.take_along_axis(d2, nbr, axis=1)
    edges = np.stack([np.repeat(np.arange(n), k), nbr.ravel(), w.ravel()], 1)
    edges = edges[np.argsort(edges[:, 2])]
    matched = np.zeros(n, dtype=bool)
    pairs = []
    for a, b, _ in edges:
        a, b = int(a), int(b)
        if not matched[a] and not matched[b]:
            matched[a] = matched[b] = True
            pairs.append((a, b))
    rest = np.nonzero(~matched)[0]
    while len(rest) > 1:
        subd = d2[np.ix_(rest, rest)]
        order = np.argsort(subd.ravel())
        used = np.zeros(len(rest), dtype=bool)
        for e in order:
            i, j = divmod(int(e), len(rest))
            if i != j and not used[i] and not used[j]:
                used[i] = used[j] = True
                pairs.append((int(rest[i]), int(rest[j])))
        rest = rest[~used]
    return np.array(pairs, dtype=np.int64)


def _cluster_bucket(r, idx, g):
    """Group r[idx] into size-g groups via hierarchical greedy pairing."""
    if g == 1:
        return [np.array([j]) for j in idx]
    cur = [np.array([j]) for j in idx]
    cents = r[idx].copy()
    while len(cur[0]) < g and len(cur) > 1:
        pairs = _greedy_pair(cents)
        newg, newc = [], []
        used = np.zeros(len(cur), dtype=bool)
        for a, b in pairs:
            na, nb = len(cur[a]), len(cur[b])
            newg.append(np.concatenate([cur[a], cur[b]]))
            newc.append((cents[a] * na + cents[b] * nb) / (na + nb))
            used[a] = used[b] = True
        for i in np.nonzero(~used)[0]:
            newg.append(cur[i])
            newc.append(cents[i])
        cur, cents = newg, np.array(newc)
    return cur


def _one_center(mem):
    """mem [ng, g, d] -> approx minimax centers [ng, d]."""
    c = mem.mean(axis=1)
    for t in range(25):
        d = np.sqrt(((mem - c[:, None, :]) ** 2).sum(-1))
        far = np.argmax(d, axis=1)
        fp = mem[np.arange(len(mem)), far]
        c = c + (1.0 / (t + 3)) * (fp - c)
    return c


def _candidate_threshold(l64, r64, lc, rc):
    """Exact (float64) max squared distance over the candidate pair set."""
    A = l64[lc]
    B = r64[rc]
    d2 = ((A * A).sum(1)[:, None] + (B * B).sum(1)[None, :]
          - 2.0 * (A @ B.T))
    return float(d2.max())


def _bf16_up(x):
    """Round x (f64 array) to bf16, forcing result >= x."""
    x = np.asarray(x, dtype=np.float64)
    y = x.astype(np.float32).astype(BF16)
    bad = y.astype(np.float64) < x
    if np.any(bad):
        bits = y.view(np.uint16)
        pos = (bits & 0x8000) == 0
        # next bf16 toward +inf: +1 ulp for positives, -1 for negatives
        # (negative zero / exact-zero handled via the pos mask on bits)
        up_bits = np.where(pos, bits + 1,
                           np.where(bits == 0x8000, np.uint16(0x0001),
                                    bits - 1)).astype(np.uint16)
        up = up_bits.view(BF16)
        y = np.where(bad, up, y)
        assert np.all(y.astype(np.float64) >= x)
    return y


def _prepare_all(l, r):
    """Returns (in_maps per core, meta dict)."""
    l64 = l.astype(np.float64)
    r64 = r.astype(np.float64)
    lsq = (l64 * l64).sum(1)
    rsq = (r64 * r64).sum(1)
    ln = np.sqrt(lsq)
    rn = np.sqrt(rsq)

    lc = _candidate_set(l64, ln)
    rc = _candidate_set(r64, rn)
    L = _candidate_threshold(l64, r64, lc, rc)

    # heat estimates for grouping / sorting (speed only, not correctness)
    m_est = _est_heat(r64, l64[lc])     # per r-point
    mu_est = _est_heat(l64, r64[rc])    # per l-row

    # group r-points: hottest get smallest groups
    order = np.argsort(m_est)[::-1]
    groups = []
    pos = 0
    for cnt, g in QUOTA:
        idx = order[pos:pos + cnt * g]
        pos += cnt * g
        groups.extend(_cluster_bucket(r64, idx, g))
    assert pos == N_R
    gsz = np.array([len(x) for x in groups])
    ng = len(groups)
    assert ng <= NG, (ng, NG)

    # group stats (vectorized per distinct size)
    cs = np.zeros((ng, DIM))
    rads = np.zeros(ng)
    ss = np.zeros(ng)
    for g in np.unique(gsz):
        sel = np.nonzero(gsz == g)[0]
        if g == 1:
            ids = [groups[i][0] for i in sel]
            cs[sel] = r64[ids]
            ss[sel] = rsq[ids]
            continue
        mem = np.stack([r64[groups[i]] for i in sel])
        c = _one_center(mem)
        cs[sel] = c
        rads[sel] = np.sqrt(((mem - c[:, None, :]) ** 2).sum(-1)).max(1)
        ss[sel] = np.stack([rsq[groups[i]] for i in sel]).max(1)

    # rigorous bf16/accum error bound for the cross term -2 l.c
    cn = np.sqrt((cs * cs).sum(1))
    delta = (2.0 ** -8) * 1.05 * (2.0 * ln.max() * max(cn.max(), 1e-9)) + 0.05
    thr = L - delta

    # device tensors.  bound slots rounded UP so device bound >= true bound
    l_aug = np.zeros((K_AUG, N_L), dtype=BF16)
    rg_aug = np.zeros((K_AUG, NG), dtype=BF16)

    # heat-sorted l (hot rows first -> flags concentrate in early chunks)
    lorder = np.argsort(mu_est)[::-1].copy()
    ls = l64[lorder]
    l_aug[:DIM] = ls.T.astype(np.float32).astype(BF16)
    l_aug[64] = _bf16_up(lsq[lorder])
    l_aug[65] = _bf16_up(ln[lorder])
    l_aug[66] = BF16(1.0)

    rg_aug[:DIM, :ng] = (-2.0 * cs.T).astype(np.float32).astype(BF16)
    rg_aug[64, :ng] = BF16(1.0)
    rg_aug[65, :ng] = _bf16_up(2.0 * rads)
    rg_aug[66, :ng] = _bf16_up(ss - thr)
    if ng < NG:  # padding groups: never flag
        rg_aug[66, ng:] = BF16(-1000.0)

    in_maps = [
        {"rg_all": np.ascontiguousarray(
            rg_aug[:, c * N_BLOCKS * 128:(c + 1) * N_BLOCKS * 128]),
         "l_all": np.ascontiguousarray(l_aug)}
        for c in range(N_CORES)
    ]
    meta = dict(groups=groups, gsz=gsz, ng=ng, L=L, thr=thr, delta=delta,
                lorder=lorder, lsq=lsq, rsq=rsq, l64=l64, r64=r64)
    return in_maps, meta


def _run_device(in_maps, nc=None):
    from concourse.bass_utils import run_bass_kernel_spmd
    if nc is None:
        nc = _get_nc()
    res = run_bass_kernel_spmd(nc, in_maps, core_ids=list(range(N_CORES)))
    return res.results


def _decode_and_recheck(results, meta):
    """Exact float64 recheck of flagged (group, l-chunk) cells."""
    groups = meta["groups"]
    lorder = meta["lorder"]
    lsq, rsq = meta["lsq"], meta["rsq"]
    l64, r64 = meta["l64"], meta["r64"]
    best = meta["L"]
    assign_act = _assignment()
    ops_per_block = LCOLS // CHUNK
    ng = meta["ng"]

    for core in range(N_CORES):
        dve = results[core].get("dve_part")
        act = results[core].get("act_part")
        dve_slot = act_slot = 0
        for op in range(N_OPS):
            if assign_act[op]:
                part = act[:, act_slot]
                act_slot += 1
            else:
                part = dve[:, dve_slot]
                dve_slot += 1
            lanes = np.nonzero(part > 0.0)[0]
            if lanes.size == 0:
                continue
            b, ch = divmod(op, ops_per_block)
            rows = lorder[ch * CHUNK:(ch + 1) * CHUNK]
            members = []
            for p in lanes:
                gid = (core * N_BLOCKS + b) * 128 + int(p)
                if gid < ng:
                    members.append(groups[gid])
            if not members:
                continue
            mem = np.concatenate(members)
            d2 = (lsq[rows][:, None] + rsq[mem][None, :]
                  - 2.0 * (l64[rows] @ r64[mem].T))
            best = max(best, float(d2.max()))
    return best


def kernel(l_dfa_embeddings, r_dfa_embeddings):
    l = np.asarray(l_dfa_embeddings, dtype=np.float32)
    r = np.asarray(r_dfa_embeddings, dtype=np.float32)
    assert l.shape == (N_L, DIM) and r.shape == (N_R, DIM)

    in_maps, meta = _prepare_all(l, r)
    results = _run_device(in_maps)
    best = _decode_and_recheck(results, meta)
    return np.float32(np.sqrt(max(best, 0.0)))


# revision 12
# speedup vs baseline: 9.7013x; 1.3424x over previous
"""Max pairwise L2 distance between two embedding sets, on 8 Trainium2 cores.

Problem: l [8192, 64] f32, r [8192, 64] f32 -> scalar f32
    out = sqrt(max_ij ||l_i - r_j||^2)

Strategy (v2: group-bound certificate)
--------------------------------------
The distance matrix has 67M entries.  On TRN2 only VectorE/ScalarE can read
PSUM (1 fp32/lane/cycle), so exhaustive per-pair examination costs ~35us.
Instead each PSUM entry certifies a whole GROUP of r-points via a provable
upper bound.  For a group G with center c, radius rad = max_j ||r_j - c||,
s = max_j ||r_j||^2:

    max_{j in G} ||l_i - r_j||^2 <= lsq_i + s - 2 l_i.c + 2 ||l_i|| rad

which is a single dot product of augmented vectors:
    rg_vec = [-2c (64) | 1 | 2*rad | s - thr]      (stationary, bf16)
    l_vec  = [l_i (64) | lsq_i | ||l_i|| | 1]      (moving, bf16)

1. Host picks a candidate max L over extreme-norm/extreme-projection pairs
   and sets thr = L - delta (delta rigorously bounds bf16 rounding).
2. Host groups r-points by "heat" (estimated max distance to any l, via the
   candidate l-set): hot points get tiny groups (size 1-2, tight bound),
   the quiet bulk gets coarse groups (4-16).  NG total groups.
3. Device: r-group vectors are the stationary operand (NG/8 = blocks of 128
   per core), all 8192 l-columns stream as moving.  ScalarE relu+accum and
   VectorE max-reduce consume PSUM; a positive partial flags (group,
   l-chunk) cells.  l-rows are heat-sorted so flags concentrate in few
   cells.
4. Host rechecks flagged cells exactly (float64) and returns
   sqrt(max(L, flagged maxima)) - an exact fp32 answer for ANY input;
   grouping quality only affects speed.
"""

import numpy as np
import ml_dtypes

N_CORES = 8
N_L, N_R, DIM = 8192, 8192, 64
K_AUG = 128                     # 64 dims + lsq/ln/1 rows + zero pad to 128
                                # (K<128 streams ~2.2x slower on the PE)
NG = 1024                       # total r-groups (multiple of 128*N_CORES)
N_BLOCKS = NG // (128 * N_CORES)
LCOLS = N_L                     # every core streams all l columns
CHUNK = 1024                    # l-cols consumed per PSUM op (legacy path)
MM_FREE = 512                   # moving free dim per matmul (1 PSUM bank)
PSUM_BUFS = 3
# op plan per block: (engine, free-dim) covering LCOLS; fds are multiples of
# 512 and each 4096-window splits ACT|DVE at a bank boundary (PE-W and
# engine-R must never share a PSUM bank).  Sized so ACT/DVE makespans
# balance: ACT (172+fd)/1.2, DVE (120+fd)/0.96.
PLAN = (("A", 2560), ("D", 1536), ("A", 2048), ("D", 2048))
N_OPS = N_BLOCKS * len(PLAN)
N_ACT = N_BLOCKS * sum(1 for e, _ in PLAN if e == "A")
# hottest r-points first: (count, group_size); counts*sizes must sum to N_R
QUOTA = ((400, 1), (8, 2), (56, 4), (176, 8), (384, 16))
BF16 = ml_dtypes.bfloat16

_COMPILED = {}


def _assignment(n_ops=N_OPS, n_act=N_ACT):
    """Bresenham-spread n_act ScalarE ops among n_ops. True = ACT."""
    out = []
    acc = 0
    for _ in range(n_ops):
        acc += n_act
        if acc >= n_ops:
            acc -= n_ops
            out.append(True)
        else:
            out.append(False)
    assert sum(out) == n_act
    return out


def _dedup_ldweights(nc):
    """Drop InstLdweights whose weights match the previous LDW in the same
    basic block (the PE keeps the loaded stationary across matmuls)."""
    removed = 0
    for fn in nc.m.functions:
        for blk in fn.blocks:
            insts = list(blk.instructions)
            last_sig = None
            keep = []
            for inst in insts:
                if type(inst).__name__ == "InstLdweights":
                    si = inst.sync_info
                    clean = si is None or (
                        not list(si.on_wait) and not list(si.on_update))
                    sig = str(inst.ins[-1])
                    if sig == last_sig and clean:
                        removed += 1
                        continue
                    last_sig = sig
                keep.append(inst)
            if len(keep) != len(insts):
                blk.instructions = keep
    return removed


def _build_nc(n_blocks=N_BLOCKS, lcols=LCOLS, chunk=CHUNK, n_act=N_ACT,
              mm_free=MM_FREE, psum_bufs=PSUM_BUFS, dyn_loop=False,
              repeats=1, dedup_ldw=True, consumer="mixed", kaug=K_AUG):
    """Build + compile the per-core SPMD program.

    Inputs : rg_all [K_AUG, n_blocks*128] bf16, l_all [K_AUG, lcols] bf16
             (+ rep_cnt [1,1] i32 when dyn_loop)
    Outputs: dve_part [128, n_dve] f32 (max of bound-thr over chunk)
             act_part [128, n_act] f32 (sum of relu(bound-thr))
    """
    import concourse.tile as tile
    from concourse import bacc, mybir
    from concourse.bass import make_scalar_value, RegisterHandles

    n_ops = n_blocks * (lcols // chunk)
    if consumer == "none":
        n_act = 0
        assign_act = [False] * n_ops
        n_dve = 1
    else:
        assign_act = _assignment(n_ops, n_act)
        n_dve = n_ops - n_act
    ngc = n_blocks * 128

    nc = bacc.Bacc("TRN2", target_bir_lowering=False, debug=False,
                   num_devices=N_CORES)
    bf16 = mybir.dt.bfloat16
    f32 = mybir.dt.float32

    rg_in = nc.dram_tensor("rg_all", [kaug, ngc], bf16,
                           kind="ExternalInput").ap()
    l_in = nc.dram_tensor("l_all", [kaug, lcols], bf16,
                          kind="ExternalInput").ap()
    cnt_in = None
    if dyn_loop:
        cnt_in = nc.dram_tensor("rep_cnt", [1, 1], mybir.dt.int32,
                                kind="ExternalInput").ap()
    dve_out = act_out = None
    if n_dve:
        dve_out = nc.dram_tensor("dve_part", [128, n_dve], f32,
                                 kind="ExternalOutput").ap()
    if n_act:
        act_out = nc.dram_tensor("act_part", [128, n_act], f32,
                                 kind="ExternalOutput").ap()

    with tile.TileContext(nc) as tc:
        with (tc.tile_pool(name="io", bufs=1) as io_pool,
              tc.tile_pool(name="psum", bufs=psum_bufs, space="PSUM") as psum_pool,
              tc.tile_pool(name="scratch", bufs=1) as scratch_pool):
            # tiny dummy activation first so the ACT table set loads during
            # the DMA prologue instead of before the first real op
            warm = scratch_pool.tile([128, 1], f32)
            nc.vector.memset(warm[:], 0.0)
            nc.scalar.activation(warm[:], warm[:],
                                 mybir.ActivationFunctionType.Relu)

            rg_sb = io_pool.tile([kaug, ngc], bf16)
            nc.sync.dma_start(rg_sb[:], rg_in[:])
            l_sb = io_pool.tile([kaug, lcols], bf16)
            # first chunk lands first so op 0 starts ASAP
            nc.sync.dma_start(l_sb[:, :chunk], l_in[:, :chunk])
            for c0 in range(chunk, lcols, chunk):
                nc.sync.dma_start(l_sb[:, c0:c0 + chunk],
                                  l_in[:, c0:c0 + chunk])

            dve_sb = act_sb = None
            if n_dve:
                dve_sb = io_pool.tile([128, n_dve], f32, name="dve_sb")
            if n_act:
                act_sb = io_pool.tile([128, n_act], f32, name="act_sb")
            if consumer == "none" and dve_sb is not None:
                nc.vector.memset(dve_sb[:], 0.0)

            def body():
                dve_slot = 0
                act_slot = 0
                op = 0
                for b in range(n_blocks):
                    stat = rg_sb[:, b * 128:(b + 1) * 128]
                    for ch in range(lcols // chunk):
                        ps = psum_pool.tile([128, chunk], f32)
                        for k in range(chunk // mm_free):
                            ncol = ch * chunk + k * mm_free
                            nc.tensor.matmul(
                                ps[:, k * mm_free:(k + 1) * mm_free],
                                stat, l_sb[:, ncol:ncol + mm_free],
                                start=True, stop=True)
                        if consumer == "none":
                            op += 1
                            continue
                        if assign_act[op]:
                            nc.scalar.activation(
                                ps[:, :], ps[:, :],
                                mybir.ActivationFunctionType.Relu,
                                accum_out=act_sb[:, act_slot:act_slot + 1])
                            act_slot += 1
                        else:
                            nc.vector.tensor_reduce(
                                dve_sb[:, dve_slot:dve_slot + 1], ps[:, :],
                                axis=mybir.AxisListType.X,
                                op=mybir.AluOpType.max)
                            dve_slot += 1
                        op += 1

            if dyn_loop:
                cnt_sb = io_pool.tile([1, 1], mybir.dt.int32)
                nc.sync.dma_start(cnt_sb[:], cnt_in[:])
                regs = []
                for etype in mybir.ALL_ENGINES:
                    eng = nc.engines[etype]
                    reg = eng.alloc_register(f"repcnt_{etype.name}")
                    eng.reg_load(reg, cnt_sb[0:1, 0:1])
                    regs.append(reg)
                end_sv = make_scalar_value(
                    RegisterHandles(regs), min_val=0, max_val=100000)
                with tc.For_i(0, end_sv):
                    for _ in range(repeats):
                        body()
            else:
                for _ in range(repeats):
                    body()

            if dve_out is not None:
                nc.sync.dma_start(dve_out[:], dve_sb[:])
            if act_out is not None:
                nc.sync.dma_start(act_out[:], act_sb[:])

    if dedup_ldw:
        _dedup_ldweights(nc)
    nc.compile()
    return nc


def _get_nc(key=("full", 1)):
    if key not in _COMPILED:
        kind, repeats = key
        _COMPILED[key] = _build_nc(repeats=repeats)
    return _COMPILED[key]


# ---------------------------------------------------------------- host side

def _candidate_set(x64, xn, nrm_top=96, nproj=24, proj_top=8, seed=777):
    """Indices of extreme-norm / extreme-projection points."""
    cs = set(np.argsort(-xn)[:nrm_top].tolist())
    rng = np.random.default_rng(seed)
    U = rng.standard_normal((nproj, DIM))
    U /= np.linalg.norm(U, axis=1, keepdims=True)
    p = x64 @ U.T
    for k in range(nproj):
        cs.update(np.argsort(-p[:, k])[:proj_top].tolist())
        cs.update(np.argsort(p[:, k])[:proj_top].tolist())
    return np.array(sorted(cs))


def _est_heat(targets, cand_pts):
    """max_i d2(cand_i, target_j) for each target row (float64)."""
    tsq = (targets * targets).sum(1)
    csq = (cand_pts * cand_pts).sum(1)
    d2 = csq[:, None] + tsq[None, :] - 2.0 * (cand_pts @ targets.T)
    return d2.max(axis=0)


def _greedy_pair(pts):
    """Greedy min-distance matching of pts [n, d] -> [n//2, 2] local idx."""
    n = pts.shape[0]
    sq = (pts * pts).sum(1)
    d2 = sq[:, None] + sq[None, :] - 2.0 * (pts @ pts.T)
    np.fill_diagonal(d2, np.inf)
    k = min(8, n - 1)
    nbr = np.argpartition(d2, k - 1, axis=1)[:, :k]
    w = np.take_along_axis(d2, nbr, axis=1)
    edges = np.stack([np.repeat(np.arange(n), k), nbr.ravel(), w.ravel()], 1)
    edges = edges[np.argsort(edges[:, 2])]
    matched = np.zeros(n, dtype=bool)
    pairs = []
    for a, b, _ in edges:
        a, b = int(a), int(b)
        if not matched[a] and not matched[b]:
            matched[a] = matched[b] = True
            pairs.append((a, b))
    rest = np.nonzero(~matched)[0]
    while len(rest) > 1:
        subd = d2[np.ix_(rest, rest)]
        order = np.argsort(subd.ravel())
        used = np.zeros(len(rest), dtype=bool)
        for e in order:
            i, j = divmod(int(e), len(rest))
            if i != j and not used[i] and not used[j]:
                used[i] = used[j] = True
                pairs.append((int(rest[i]), int(rest[j])))
        rest = rest[~used]
    return np.array(pairs, dtype=np.int64)


def _cluster_bucket(r, idx, g):
    """Group r[idx] into size-g groups via hierarchical greedy pairing."""
    if g == 1:
        return [np.array([j]) for j in idx]
    cur = [np.array([j]) for j in idx]
    cents = r[idx].copy()
    while len(cur[0]) < g and len(cur) > 1:
        pairs = _greedy_pair(cents)
        newg, newc = [], []
        used = np.zeros(len(cur), dtype=bool)
        for a, b in pairs:
            na, nb = len(cur[a]), len(cur[b])
            newg.append(np.concatenate([cur[a], cur[b]]))
            newc.append((cents[a] * na + cents[b] * nb) / (na + nb))
            used[a] = used[b] = True
        for i in np.nonzero(~used)[0]:
            newg.append(cur[i])
            newc.append(cents[i])
        cur, cents = newg, np.array(newc)
    return cur


def _one_center(mem):
    """mem [ng, g, d] -> approx minimax centers [ng, d]."""
    c = mem.mean(axis=1)
    for t in range(25):
        d = np.sqrt(((mem - c[:, None, :]) ** 2).sum(-1))
        far = np.argmax(d, axis=1)
        fp = mem[np.arange(len(mem)), far]
        c = c + (1.0 / (t + 3)) * (fp - c)
    return c


def _candidate_threshold(l64, r64, lc, rc):
    """Exact (float64) max squared distance over the candidate pair set."""
    A = l64[lc]
    B = r64[rc]
    d2 = ((A * A).sum(1)[:, None] + (B * B).sum(1)[None, :]
          - 2.0 * (A @ B.T))
    return float(d2.max())


def _bf16_up(x):
    """Round x (f64 array) to bf16, forcing result >= x."""
    x = np.asarray(x, dtype=np.float64)
    y = x.astype(np.float32).astype(BF16)
    bad = y.astype(np.float64) < x
    if np.any(bad):
        bits = y.view(np.uint16)
        pos = (bits & 0x8000) == 0
        # next bf16 toward +inf: +1 ulp for positives, -1 for negatives
        # (negative zero / exact-zero handled via the pos mask on bits)
        up_bits = np.where(pos, bits + 1,
                           np.where(bits == 0x8000, np.uint16(0x0001),
                                    bits - 1)).astype(np.uint16)
        up = up_bits.view(BF16)
        y = np.where(bad, up, y)
        assert np.all(y.astype(np.float64) >= x)
    return y


def _prepare_all(l, r):
    """Returns (in_maps per core, meta dict)."""
    l64 = l.astype(np.float64)
    r64 = r.astype(np.float64)
    lsq = (l64 * l64).sum(1)
    rsq = (r64 * r64).sum(1)
    ln = np.sqrt(lsq)
    rn = np.sqrt(rsq)

    lc = _candidate_set(l64, ln)
    rc = _candidate_set(r64, rn)
    L = _candidate_threshold(l64, r64, lc, rc)

    # heat estimates for grouping / sorting (speed only, not correctness)
    m_est = _est_heat(r64, l64[lc])     # per r-point
    mu_est = _est_heat(l64, r64[rc])    # per l-row

    # group r-points: hottest get smallest groups
    order = np.argsort(m_est)[::-1]
    groups = []
    pos = 0
    for cnt, g in QUOTA:
        idx = order[pos:pos + cnt * g]
        pos += cnt * g
        groups.extend(_cluster_bucket(r64, idx, g))
    assert pos == N_R
    gsz = np.array([len(x) for x in groups])
    ng = len(groups)
    assert ng <= NG, (ng, NG)

    # group stats (vectorized per distinct size)
    cs = np.zeros((ng, DIM))
    rads = np.zeros(ng)
    ss = np.zeros(ng)
    for g in np.unique(gsz):
        sel = np.nonzero(gsz == g)[0]
        if g == 1:
            ids = [groups[i][0] for i in sel]
            cs[sel] = r64[ids]
            ss[sel] = rsq[ids]
            continue
        mem = np.stack([r64[groups[i]] for i in sel])
        c = _one_center(mem)
        cs[sel] = c
        rads[sel] = np.sqrt(((mem - c[:, None, :]) ** 2).sum(-1)).max(1)
        ss[sel] = np.stack([rsq[groups[i]] for i in sel]).max(1)

    # rigorous bf16/accum error bound for the cross term -2 l.c
    cn = np.sqrt((cs * cs).sum(1))
    delta = (2.0 ** -8) * 1.05 * (2.0 * ln.max() * max(cn.max(), 1e-9)) + 0.05
    thr = L - delta

    # device tensors.  bound slots rounded UP so device bound >= true bound
    l_aug = np.zeros((K_AUG, N_L), dtype=BF16)
    rg_aug = np.zeros((K_AUG, NG), dtype=BF16)

    # heat-sorted l (hot rows first -> flags concentrate in early chunks)
    lorder = np.argsort(mu_est)[::-1].copy()
    ls = l64[lorder]
    l_aug[:DIM] = ls.T.astype(np.float32).astype(BF16)
    l_aug[64] = _bf16_up(lsq[lorder])
    l_aug[65] = _bf16_up(ln[lorder])
    l_aug[66] = BF16(1.0)

    rg_aug[:DIM, :ng] = (-2.0 * cs.T).astype(np.float32).astype(BF16)
    rg_aug[64, :ng] = BF16(1.0)
    rg_aug[65, :ng] = _bf16_up(2.0 * rads)
    rg_aug[66, :ng] = _bf16_up(ss - thr)
    if ng < NG:  # padding groups: never flag
        rg_aug[66, ng:] = BF16(-1000.0)

    in_maps = [
        {"rg_all": np.ascontiguousarray(
            rg_aug[:, c * N_BLOCKS * 128:(c + 1) * N_BLOCKS * 128]),
         "l_all": np.ascontiguousarray(l_aug)}
        for c in range(N_CORES)
    ]
    meta = dict(groups=groups, gsz=gsz, ng=ng, L=L, thr=thr, delta=delta,
                lorder=lorder, lsq=lsq, rsq=rsq, l64=l64, r64=r64)
    return in_maps, meta


def _run_device(in_maps, nc=None):
    from concourse.bass_utils import run_bass_kernel_spmd
    if nc is None:
        nc = _get_nc()
    res = run_bass_kernel_spmd(nc, in_maps, core_ids=list(range(N_CORES)))
    return res.results


def _decode_and_recheck(results, meta):
    """Exact float64 recheck of flagged (group, l-chunk) cells."""
    groups = meta["groups"]
    lorder = meta["lorder"]
    lsq, rsq = meta["lsq"], meta["rsq"]
    l64, r64 = meta["l64"], meta["r64"]
    best = meta["L"]
    assign_act = _assignment()
    ops_per_block = LCOLS // CHUNK
    ng = meta["ng"]

    for core in range(N_CORES):
        dve = results[core].get("dve_part")
        act = results[core].get("act_part")
        dve_slot = act_slot = 0
        for op in range(N_OPS):
            if assign_act[op]:
                part = act[:, act_slot]
                act_slot += 1
            else:
                part = dve[:, dve_slot]
                dve_slot += 1
            lanes = np.nonzero(part > 0.0)[0]
            if lanes.size == 0:
                continue
            b, ch = divmod(op, ops_per_block)
            rows = lorder[ch * CHUNK:(ch + 1) * CHUNK]
            members = []
            for p in lanes:
                gid = (core * N_BLOCKS + b) * 128 + int(p)
                if gid < ng:
                    members.append(groups[gid])
            if not members:
                continue
            mem = np.concatenate(members)
            d2 = (lsq[rows][:, None] + rsq[mem][None, :]
                  - 2.0 * (l64[rows] @ r64[mem].T))
            best = max(best, float(d2.max()))
    return best


def kernel(l_dfa_embeddings, r_dfa_embeddings):
    l = np.asarray(l_dfa_embeddings, dtype=np.float32)
    r = np.asarray(r_dfa_embeddings, dtype=np.float32)
    assert l.shape == (N_L, DIM) and r.shape == (N_R, DIM)

    in_maps, meta = _prepare_all(l, r)
    results = _run_device(in_maps)
    best = _decode_and_recheck(results, meta)
    return np.float32(np.sqrt(max(best, 0.0)))
